# revision 43
# baseline (speedup 1.0000x reference)
"""GumbelSlotSelector Trainium kernel (f32r matmuls, 4KB-chunk DMA).

Math (per row r of B*K rows, D=128, H=64):
  h = relu(x @ W1 + b1);  dlogit = h @ (W2[:,1]-W2[:,0]) + (b2[1]-b2[0])
  decision = 1.0 if dlogit + g1 - g0 > 0 else 0.0,  g_i = -log(-log(clip(u_i)))
  keep_probs = sigmoid(dlogit)
  fixup: rows (of K=64 slots) with no active slot activate their argmax(fix_u) slot.

Sharding: pure data-parallel over batch B=8192 -> 8 cores x 1024 rows
(65536 (b,k)-rows of 128 features per core).

Per-core dataflow (strips of 1024 rows), all matmuls float32r:
  x strip loaded as x_sb[p, (t d)] = row 8p+t  (one contiguous 4KB
  chunk per partition -> 8x fewer DMA descriptors than (t p) order).
  8 PE transposes (two half-strip psum tiles) -> xt_sb[d, 128t+p]
  (block column order, row 8p+t at column c=128t+p).
  mm1 per half -> ht[h', c] in block order.
  relu+bias reads ht through a permuting strided AP so that
  relu_sb[64e+h', j] = relu row 512e+j  (j=8*jh+jl <- c=128*jl+64e+jh),
  packing the two 512-row halves into partitions 0-63 / 64-127.
  mm2: lhsT = 64-col shifted view of a [128, 126] w2d pattern ->
  dl[2s'+e, j] += w2d . relu  accumulated over 32 strips per half,
  giving dl[p, j] = dlogit(row 512p + j) per half.
  Final gumbel + fixup phase per half overlaps the other half's strips.
"""
import sys

sys.path.insert(0, "/opt/trn_rl_repo")
import numpy as np
from contextlib import ExitStack

import concourse.bacc as bacc
import concourse.tile as tile
from concourse import mybir, bass_utils
from concourse.bass_interp import get_hw_module

F32 = mybir.dt.float32
F32R = mybir.dt.float32r
AF = mybir.ActivationFunctionType
ALU = mybir.AluOpType

B, K, D, H = 8192, 64, 128, 64
NCORES = 8
R = (B // NCORES) * K          # 65536 rows per core
SR = 1024                      # strip rows
NSTRIP = R // SR               # 64
HALF = R // 2                  # 32768 rows per dl half
CLIP_LO = 1e-10
CLIP_HI = float(np.float32(1.0 - 1e-7))

_CACHE = {}


def _build():
    nc = bacc.Bacc("TRN2", target_bir_lowering=False, debug=False,
                   num_devices=NCORES)
    x_d = nc.dram_tensor("x", [R, D], F32R, kind="ExternalInput")
    gu_d = nc.dram_tensor("gu", [R, 2], F32, kind="ExternalInput")
    fu_d = nc.dram_tensor("fu", [R], F32, kind="ExternalInput")
    w1_d = nc.dram_tensor("w1", [D, H], F32R, kind="ExternalInput")
    w2p_d = nc.dram_tensor("w2p", [128, 126], F32R, kind="ExternalInput")
    b1_d = nc.dram_tensor("b1c", [H, 1], F32, kind="ExternalInput")
    b2_d = nc.dram_tensor("b2dv", [H, 1], F32, kind="ExternalInput")
    eye_d = nc.dram_tensor("eye", [128, 128], F32R, kind="ExternalInput")
    dec_d = nc.dram_tensor("dec", [R], F32, kind="ExternalOutput")
    keep_d = nc.dram_tensor("keep", [R], F32, kind="ExternalOutput")

    with tile.TileContext(nc) as tc, ExitStack() as ctx:
        cpool = ctx.enter_context(tc.tile_pool(name="const", bufs=1))
        xpool = ctx.enter_context(tc.tile_pool(name="x", bufs=1))
        tpool = ctx.enter_context(tc.tile_pool(name="xt", bufs=1))
        rpool = ctx.enter_context(tc.tile_pool(name="relu", bufs=1))
        gpool = ctx.enter_context(tc.tile_pool(name="gin", bufs=1))
        fpool = ctx.enter_context(tc.tile_pool(name="fin", bufs=1))
        ps_xt = ctx.enter_context(tc.tile_pool(name="psxt", bufs=1, space="PSUM"))
        ps_ht = ctx.enter_context(tc.tile_pool(name="psht", bufs=2, space="PSUM"))
        ps_dl = ctx.enter_context(tc.tile_pool(name="psdl", bufs=1, space="PSUM"))

        dl_ps = [ps_dl.tile([64, 512], F32, name=f"dl{i}") for i in range(2)]
        gu_sb = [None, None]
        fu_sb = [None, None]
        tg_sb = [None, None]
        fmx_sb = [None, None]
        w1_sb = w2p_sb = b1_sb = b2_sb = eye_sb = None

        def load_consts():
            nonlocal w1_sb, w2p_sb, b1_sb, b2_sb, eye_sb
            eye_sb = cpool.tile([128, 128], F32R)
            nc.sync.dma_start(eye_sb[:], eye_d.ap())
            w1_sb = cpool.tile([D, H], F32R)
            nc.sync.dma_start(w1_sb[:], w1_d.ap())
            w2p_sb = cpool.tile([128, 126], F32R)
            nc.sync.dma_start(w2p_sb[:], w2p_d.ap())
            b1_sb = cpool.tile([H, 1], F32)
            nc.sync.dma_start(b1_sb[:], b1_d.ap())
            b2_sb = cpool.tile([H, 1], F32)
            nc.sync.dma_start(b2_sb[:], b2_d.ap())

        def load_half_inputs(hf):
            off = hf * HALF
            gu_sb[hf] = gpool.tile([64, 1024], F32, name=f"gu{hf}")
            nc.gpsimd.dma_start(
                gu_sb[hf][:].rearrange("p (s u) -> p s u", u=2),
                gu_d.ap()[off:off + HALF, :].rearrange("(p s) u -> p s u", p=64),
            )
            fu_sb[hf] = gpool.tile([64, 512], F32, name=f"fu{hf}")
            nc.gpsimd.dma_start(
                fu_sb[hf][:],
                fu_d.ap()[off:off + HALF].rearrange("(p s) -> p s", p=64))

        def prep_half(hf):
            # everything that does not depend on dl: gumbel threshold
            # tg = -(g1 - g0) - b2  (dec = dl > tg), and fmx = rowmax(fix_u)
            gu_v = gu_sb[hf][:].rearrange("p (s u) -> p s u", u=2)
            a0 = fpool.tile([64, 512], F32, name=f"a0{hf}")
            a1 = fpool.tile([64, 512], F32, name=f"a1{hf}")
            nc.vector.tensor_scalar(a0[:], gu_v[:, :, 0], CLIP_LO, CLIP_HI,
                                    op0=ALU.max, op1=ALU.min)
            nc.vector.tensor_scalar(a1[:], gu_v[:, :, 1], CLIP_LO, CLIP_HI,
                                    op0=ALU.max, op1=ALU.min)
            # g_i = -log(-log(u_i)); g0m = log(-log u0) = -g0
            nc.scalar.activation(a0[:], a0[:], AF.Ln)
            nc.scalar.activation(a1[:], a1[:], AF.Ln)
            g0m = fpool.tile([64, 512], F32, name=f"g0m{hf}")
            g1m = fpool.tile([64, 512], F32, name=f"g1m{hf}")
            nc.scalar.activation(g0m[:], a0[:], AF.Ln, scale=-1.0)
            nc.scalar.activation(g1m[:], a1[:], AF.Ln, scale=-1.0)
            # tg = (g1m - g0m) - b2 = -(g1 - g0) - b2
            tg_sb[hf] = fpool.tile([64, 512], F32, name=f"tg{hf}")
            nc.vector.scalar_tensor_tensor(
                tg_sb[hf][:], g1m[:], b2_sb[:, 0:1], g0m[:],
                op0=ALU.subtract, op1=ALU.subtract)
            fu_v = fu_sb[hf][:].rearrange("p (g k) -> p g k", k=64)
            fmx_sb[hf] = fpool.tile([64, 8], F32, name=f"fmx{hf}")
            nc.vector.reduce_max(fmx_sb[hf][:], fu_v, axis=mybir.AxisListType.X)

        def final_half_ops(hf):
            """Return thunks for the post-accumulation phase; issue in order."""
            off = hf * HALF
            dl = dl_ps[hf]
            dec_sb = fpool.tile([64, 512], F32, name=f"dec{hf}")
            keep_sb = fpool.tile([64, 512], F32, name=f"keep{hf}")
            rs = fpool.tile([64, 8], F32, name=f"rs{hf}")
            nn = fpool.tile([64, 8], F32, name=f"nn{hf}")
            fmxx = fpool.tile([64, 8], F32, name=f"fmxx{hf}")
            fixm = fpool.tile([64, 512], F32, name=f"fixm{hf}")
            dec_v = dec_sb[:].rearrange("p (g k) -> p g k", k=64)
            fu_v = fu_sb[hf][:].rearrange("p (g k) -> p g k", k=64)

            # fixup: rows with no active slot activate argmax(fix_u).
            # fmxx = fmx + BIG*min(active_count, 1): fu >= fmxx only hits
            # the argmax slot of all-inactive rows.
            return [
                lambda: (
                    nc.vector.tensor_tensor(dec_sb[:], dl[:], tg_sb[hf][:],
                                            op=ALU.is_gt),
                    nc.scalar.activation(keep_sb[:], dl[:], AF.Sigmoid,
                                         bias=b2_sb[:, 0:1]),
                ),
                lambda: nc.vector.reduce_sum(rs[:], dec_v,
                                             axis=mybir.AxisListType.X),
                lambda: (
                    nc.vector.tensor_scalar(nn[:], rs[:], 1.0, 1.0e9,
                                            op0=ALU.min, op1=ALU.mult),
                    nc.vector.tensor_tensor(fmxx[:], nn[:], fmx_sb[hf][:],
                                            op=ALU.add),
                ),
                lambda: nc.vector.tensor_tensor(
                    fixm[:].rearrange("p (g k) -> p g k", k=64), fu_v,
                    fmxx[:].rearrange("p (g o) -> p g o", o=1).broadcast_to(
                        [64, 8, 64]),
                    op=ALU.is_ge),
                lambda: nc.vector.tensor_tensor(dec_sb[:], dec_sb[:], fixm[:],
                                                op=ALU.max),
                lambda: nc.gpsimd.dma_start(
                    dec_d.ap()[off:off + HALF].rearrange("(p s) -> p s", p=64),
                    dec_sb[:]),
                lambda: nc.gpsimd.dma_start(
                    keep_d.ap()[off:off + HALF].rearrange("(p s) -> p s", p=64),
                    keep_sb[:]),
            ]

        # software pipeline: iteration `it` issues DMA(it), transposes+
        # copies(it-2), mm1/relu(it-3), mm2(it-4) so each engine's static
        # queue has stage N of strip s ahead of stage N+1 of strip s-1,
        # and the PE never waits on relu for mm2 of the same strip.
        xt_sbs = {}
        relu_sbs = {}
        pending_ops = []

        for it in range(NSTRIP + 4):
            if it < NSTRIP:
                s = it
                # x_sb[p, t*128+d] = x[strip row 8p+t, d]: 2KB chunks,
                # two half dma_starts so transposes can begin earlier
                x_sb = xpool.tile([128, SR], F32R, name=f"x{s % 13}")
                xt_sbs[s] = (x_sb, None)
                if s < 16:
                    # ramp: one dispatch per strip fills the DMA queue
                    # faster; the first two go out on the gpsimd queue in
                    # parallel with the sync queue's const dispatches
                    eng = nc.gpsimd if s < 2 else nc.sync
                    eng.dma_start(
                        x_sb[:].rearrange("p (t d) -> p t d", d=D),
                        x_d.ap()[s * SR:(s + 1) * SR, :].rearrange(
                            "(p t) d -> p t d", p=128),
                    )
                else:
                    for h in range(2):
                        nc.sync.dma_start(
                            x_sb[:, 512 * h:512 * (h + 1)].rearrange(
                                "p (t d) -> p t d", d=D),
                            x_d.ap()[s * SR:(s + 1) * SR, :].rearrange(
                                "(p t) d -> p t d", p=128)[:, 4 * h:4 * (h + 1), :],
                        )
                if s == 0:
                    load_consts()
                elif s == 4:
                    load_half_inputs(0)
                elif s == 6:
                    prep_half(0)
                elif s == 8:
                    load_half_inputs(1)
                elif s == 10:
                    prep_half(1)
            if 2 <= it < NSTRIP + 2:
                s = it - 2
                x_sb = xt_sbs[s][0]
                # xt_sb[d, 128t+p] = row 8p+t; two half-strip psum tiles
                xt_sb = tpool.tile([128, SR], F32R, name=f"xt{s % 5}")
                xt_sbs[s] = (x_sb, xt_sb)
                for h in range(2):
                    xt_ps = ps_xt.tile([128, 512], F32R, name=f"xtp{h}")
                    for tq in range(4):
                        t = 4 * h + tq
                        nc.tensor.transpose(
                            xt_ps[:, tq * 128:(tq + 1) * 128],
                            x_sb[:, t * D:(t + 1) * D],
                            eye_sb[:],
                        )
                    if h == 0:
                        nc.vector.tensor_copy(xt_sb[:, 0:512], xt_ps[:])
                    else:
                        nc.scalar.copy(xt_sb[:, 512:1024], xt_ps[:])
            if 3 <= it < NSTRIP + 3:
                s = it - 3
                xt_sb = xt_sbs.pop(s)[1]
                ht_ps = ps_ht.tile([H, SR], F32)
                for k in range(2):
                    nc.tensor.matmul(
                        ht_ps[:, k * 512:(k + 1) * 512],
                        w1_sb[:],
                        xt_sb[:, k * 512:(k + 1) * 512],
                        start=True, stop=True,
                    )
                # relu + permute to natural row order, packed [128, 512]:
                # relu_sb[64e+h', 8jh+jl] = relu(ht[h', 128jl+64e+jh] + b1)
                relu_sb = rpool.tile([128, 512], F32R, name=f"r{s % 5}")
                relu_sbs[s] = relu_sb
                ht_v = ht_ps[:].rearrange("h (jl pp) -> h pp jl", pp=128)
                nc.vector.tensor_scalar(
                    relu_sb[0:64, :].rearrange("h (jh jl) -> h jh jl", jl=8),
                    ht_v[:, 0:64, :], b1_sb[:, 0:1], 0.0,
                    op0=ALU.add, op1=ALU.max)
                nc.scalar.activation(
                    relu_sb[64:128, :].rearrange("h (jh jl) -> h jh jl", jl=8),
                    ht_v[:, 64:128, :], AF.Relu, bias=b1_sb[:, 0:1])
            if 4 <= it:
                s = it - 4
                hf, sp = s // 32, s % 32
                relu_sb = relu_sbs.pop(s)
                # dl[2*sp+e, j] += w2d . relu_half_e[:, j]
                nc.tensor.matmul(
                    dl_ps[hf][:],
                    w2p_sb[:, 62 - 2 * sp:126 - 2 * sp],
                    relu_sb[:],
                    start=(sp == 0), stop=(sp == 31),
                    skip_group_check=True,
                )
                if s == 31:
                    pending_ops.extend(final_half_ops(0))
                if pending_ops:
                    pending_ops.pop(0)()
        for op in pending_ops:
            op()
        for op in final_half_ops(1):
            op()

    nc.compile()
    nc.m = get_hw_module(nc.m)
    return nc


def kernel(slots, gumbel_u, fix_u, W1, b1, W2, b2, _trace=False):
    slots = np.ascontiguousarray(slots, np.float32)
    gumbel_u = np.ascontiguousarray(gumbel_u, np.float32)
    fix_u = np.ascontiguousarray(fix_u, np.float32)
    W1 = np.ascontiguousarray(W1, np.float32)
    W2 = np.ascontiguousarray(W2, np.float32)
    w2d = (W2[:, 1] - W2[:, 0]).astype(np.float32)
    b2d = np.float32(b2[1] - b2[0])

    w2p = np.zeros((128, 126), np.float32)
    w2p[0:64, 62] = w2d
    w2p[64:128, 63] = w2d
    b1c = np.ascontiguousarray(b1, np.float32).reshape(H, 1)
    b2dv = np.full((H, 1), b2d, np.float32)
    eye = np.eye(128, dtype=np.float32)

    if "nc" not in _CACHE:
        _CACHE["nc"] = _build()
    nc = _CACHE["nc"]

    bpc = B // NCORES
    in_maps = []
    for c in range(NCORES):
        in_maps.append({
            "x": slots[c * bpc:(c + 1) * bpc].reshape(R, D),
            "gu": gumbel_u[c * bpc:(c + 1) * bpc].reshape(R, 2),
            "fu": fix_u[c * bpc:(c + 1) * bpc].reshape(R),
            "w1": W1, "w2p": w2p, "b1c": b1c, "b2dv": b2dv, "eye": eye,
        })
    res = bass_utils.run_bass_kernel_spmd(
        nc, in_maps, core_ids=list(range(NCORES)), trace=_trace)
    _CACHE["last_result"] = res

    dec = np.concatenate(
        [res.results[c]["dec"].reshape(bpc, K) for c in range(NCORES)], axis=0)
    keep = np.concatenate(
        [res.results[c]["keep"].reshape(bpc, K) for c in range(NCORES)], axis=0)
    return dec, keep


# revision 44
# speedup vs baseline: 1.0609x; 1.0609x over previous
"""GumbelSlotSelector Trainium kernel (f32r matmuls, 4KB-chunk DMA).

Math (per row r of B*K rows, D=128, H=64):
  h = relu(x @ W1 + b1);  dlogit = h @ (W2[:,1]-W2[:,0]) + (b2[1]-b2[0])
  decision = 1.0 if dlogit + g1 - g0 > 0 else 0.0,  g_i = -log(-log(clip(u_i)))
  keep_probs = sigmoid(dlogit)
  fixup: rows (of K=64 slots) with no active slot activate their argmax(fix_u) slot.

Sharding: pure data-parallel over batch B=8192 -> 8 cores x 1024 rows
(65536 (b,k)-rows of 128 features per core).

Per-core dataflow (strips of 1024 rows), all matmuls float32r:
  x strip loaded as x_sb[p, (t d)] = row 8p+t  (one contiguous 4KB
  chunk per partition -> 8x fewer DMA descriptors than (t p) order).
  8 PE transposes (two half-strip psum tiles) -> xt_sb[d, 128t+p]
  (block column order, row 8p+t at column c=128t+p).
  mm1 per half -> ht[h', c] in block order.
  relu+bias reads ht through a permuting strided AP so that
  relu_sb[64e+h', j] = relu row 512e+j  (j=8*jh+jl <- c=128*jl+64e+jh),
  packing the two 512-row halves into partitions 0-63 / 64-127.
  mm2: lhsT = 64-col shifted view of a [128, 126] w2d pattern ->
  dl[2s'+e, j] += w2d . relu  accumulated over 32 strips per half,
  giving dl[p, j] = dlogit(row 512p + j) per half.
  Final gumbel + fixup phase per half overlaps the other half's strips.
"""
import sys

sys.path.insert(0, "/opt/trn_rl_repo")
import numpy as np
from contextlib import ExitStack

import concourse.bacc as bacc
import concourse.tile as tile
from concourse import mybir, bass_utils
from concourse.bass_interp import get_hw_module

F32 = mybir.dt.float32
F32R = mybir.dt.float32r
AF = mybir.ActivationFunctionType
ALU = mybir.AluOpType

B, K, D, H = 8192, 64, 128, 64
NCORES = 8
R = (B // NCORES) * K          # 65536 rows per core
SR = 1024                      # strip rows
NSTRIP = R // SR               # 64
HALF = R // 2                  # 32768 rows per dl half
CLIP_LO = 1e-10
CLIP_HI = float(np.float32(1.0 - 1e-7))

_CACHE = {}


def _build():
    nc = bacc.Bacc("TRN2", target_bir_lowering=False, debug=False,
                   num_devices=NCORES)
    x_d = nc.dram_tensor("x", [R, D], F32R, kind="ExternalInput")
    gu_d = nc.dram_tensor("gu", [R, 2], F32, kind="ExternalInput")
    fu_d = nc.dram_tensor("fu", [R], F32, kind="ExternalInput")
    w1_d = nc.dram_tensor("w1", [D, H], F32R, kind="ExternalInput")
    w2p_d = nc.dram_tensor("w2p", [128, 126], F32R, kind="ExternalInput")
    b1_d = nc.dram_tensor("b1c", [H, 1], F32, kind="ExternalInput")
    b2_d = nc.dram_tensor("b2dv", [H, 1], F32, kind="ExternalInput")
    eye_d = nc.dram_tensor("eye", [128, 128], F32R, kind="ExternalInput")
    dec_d = nc.dram_tensor("dec", [R], F32, kind="ExternalOutput")
    keep_d = nc.dram_tensor("keep", [R], F32, kind="ExternalOutput")

    with tile.TileContext(nc) as tc, ExitStack() as ctx:
        cpool = ctx.enter_context(tc.tile_pool(name="const", bufs=1))
        xpool = ctx.enter_context(tc.tile_pool(name="x", bufs=1))
        tpool = ctx.enter_context(tc.tile_pool(name="xt", bufs=1))
        rpool = ctx.enter_context(tc.tile_pool(name="relu", bufs=1))
        gpool = ctx.enter_context(tc.tile_pool(name="gin", bufs=1))
        fpool = ctx.enter_context(tc.tile_pool(name="fin", bufs=1))
        ps_xt = ctx.enter_context(tc.tile_pool(name="psxt", bufs=1, space="PSUM"))
        ps_ht = ctx.enter_context(tc.tile_pool(name="psht", bufs=2, space="PSUM"))
        ps_dl = ctx.enter_context(tc.tile_pool(name="psdl", bufs=1, space="PSUM"))

        dl_ps = [ps_dl.tile([64, 512], F32, name=f"dl{i}") for i in range(2)]
        gu_sb = [None, None]
        fu_sb = [None, None]
        tg_sb = [None, None]
        fmx_sb = [None, None]
        w1_sb = w2p_sb = b1_sb = b2_sb = eye_sb = None

        def load_consts():
            nonlocal w1_sb, w2p_sb, b1_sb, b2_sb, eye_sb
            eye_sb = cpool.tile([128, 128], F32R)
            nc.sync.dma_start(eye_sb[:], eye_d.ap())
            w1_sb = cpool.tile([D, H], F32R)
            nc.sync.dma_start(w1_sb[:], w1_d.ap())
            w2p_sb = cpool.tile([128, 126], F32R)
            nc.sync.dma_start(w2p_sb[:], w2p_d.ap())
            b1_sb = cpool.tile([H, 1], F32)
            nc.sync.dma_start(b1_sb[:], b1_d.ap())
            b2_sb = cpool.tile([H, 1], F32)
            nc.sync.dma_start(b2_sb[:], b2_d.ap())

        def load_half_inputs(hf):
            off = hf * HALF
            gu_sb[hf] = gpool.tile([64, 1024], F32, name=f"gu{hf}")
            nc.gpsimd.dma_start(
                gu_sb[hf][:].rearrange("p (s u) -> p s u", u=2),
                gu_d.ap()[off:off + HALF, :].rearrange("(p s) u -> p s u", p=64),
            )
            fu_sb[hf] = gpool.tile([64, 512], F32, name=f"fu{hf}")
            nc.gpsimd.dma_start(
                fu_sb[hf][:],
                fu_d.ap()[off:off + HALF].rearrange("(p s) -> p s", p=64))

        def prep_half(hf):
            # everything that does not depend on dl: gumbel threshold
            # tg = -(g1 - g0) - b2  (dec = dl > tg), and fmx = rowmax(fix_u)
            gu_v = gu_sb[hf][:].rearrange("p (s u) -> p s u", u=2)
            a0 = fpool.tile([64, 512], F32, name=f"a0{hf}")
            a1 = fpool.tile([64, 512], F32, name=f"a1{hf}")
            nc.vector.tensor_scalar(a0[:], gu_v[:, :, 0], CLIP_LO, CLIP_HI,
                                    op0=ALU.max, op1=ALU.min)
            nc.vector.tensor_scalar(a1[:], gu_v[:, :, 1], CLIP_LO, CLIP_HI,
                                    op0=ALU.max, op1=ALU.min)
            # g_i = -log(-log(u_i)); g0m = log(-log u0) = -g0
            nc.scalar.activation(a0[:], a0[:], AF.Ln)
            nc.scalar.activation(a1[:], a1[:], AF.Ln)
            g0m = fpool.tile([64, 512], F32, name=f"g0m{hf}")
            g1m = fpool.tile([64, 512], F32, name=f"g1m{hf}")
            nc.scalar.activation(g0m[:], a0[:], AF.Ln, scale=-1.0)
            nc.scalar.activation(g1m[:], a1[:], AF.Ln, scale=-1.0)
            # tg = (g1m - g0m) - b2 = -(g1 - g0) - b2
            tg_sb[hf] = fpool.tile([64, 512], F32, name=f"tg{hf}")
            nc.vector.scalar_tensor_tensor(
                tg_sb[hf][:], g1m[:], b2_sb[:, 0:1], g0m[:],
                op0=ALU.subtract, op1=ALU.subtract)
            fu_v = fu_sb[hf][:].rearrange("p (g k) -> p g k", k=64)
            fmx_sb[hf] = fpool.tile([64, 8], F32, name=f"fmx{hf}")
            nc.vector.reduce_max(fmx_sb[hf][:], fu_v, axis=mybir.AxisListType.X)

        def final_half_ops(hf):
            """Return thunks for the post-accumulation phase; issue in order."""
            off = hf * HALF
            dl = dl_ps[hf]
            dec_sb = fpool.tile([64, 512], F32, name=f"dec{hf}")
            keep_sb = fpool.tile([64, 512], F32, name=f"keep{hf}")
            rs = fpool.tile([64, 8], F32, name=f"rs{hf}")
            nn = fpool.tile([64, 8], F32, name=f"nn{hf}")
            fmxx = fpool.tile([64, 8], F32, name=f"fmxx{hf}")
            fixm = fpool.tile([64, 512], F32, name=f"fixm{hf}")
            dec_v = dec_sb[:].rearrange("p (g k) -> p g k", k=64)
            fu_v = fu_sb[hf][:].rearrange("p (g k) -> p g k", k=64)

            # fixup: rows with no active slot activate argmax(fix_u).
            # fmxx = fmx + BIG*min(active_count, 1): fu >= fmxx only hits
            # the argmax slot of all-inactive rows.
            return [
                lambda: (
                    nc.vector.tensor_tensor(dec_sb[:], dl[:], tg_sb[hf][:],
                                            op=ALU.is_gt),
                    nc.scalar.activation(keep_sb[:], dl[:], AF.Sigmoid,
                                         bias=b2_sb[:, 0:1]),
                ),
                lambda: nc.vector.reduce_sum(rs[:], dec_v,
                                             axis=mybir.AxisListType.X),
                lambda: (
                    nc.vector.tensor_scalar(nn[:], rs[:], 1.0, 1.0e9,
                                            op0=ALU.min, op1=ALU.mult),
                    nc.vector.tensor_tensor(fmxx[:], nn[:], fmx_sb[hf][:],
                                            op=ALU.add),
                ),
                lambda: nc.vector.tensor_tensor(
                    fixm[:].rearrange("p (g k) -> p g k", k=64), fu_v,
                    fmxx[:].rearrange("p (g o) -> p g o", o=1).broadcast_to(
                        [64, 8, 64]),
                    op=ALU.is_ge),
                lambda: nc.vector.tensor_tensor(dec_sb[:], dec_sb[:], fixm[:],
                                                op=ALU.max),
                lambda: nc.gpsimd.dma_start(
                    dec_d.ap()[off:off + HALF].rearrange("(p s) -> p s", p=64),
                    dec_sb[:]),
                lambda: nc.gpsimd.dma_start(
                    keep_d.ap()[off:off + HALF].rearrange("(p s) -> p s", p=64),
                    keep_sb[:]),
            ]

        # software pipeline: iteration `it` issues DMA(it), transposes+
        # copies(it-2), mm1/relu(it-3), mm2(it-4) so each engine's static
        # queue has stage N of strip s ahead of stage N+1 of strip s-1,
        # and the PE never waits on relu for mm2 of the same strip.
        xt_sbs = {}
        relu_sbs = {}
        pending_ops = []

        for it in range(NSTRIP + 4):
            if it < NSTRIP:
                s = it
                # x_sb[p, t*128+d] = x[strip row 8p+t, d]: 2KB chunks,
                # two half dma_starts so transposes can begin earlier
                x_sb = xpool.tile([128, SR], F32R, name=f"x{s % 13}")
                xt_sbs[s] = (x_sb, None)
                if s < 16:
                    # ramp: one dispatch per strip fills the DMA queue faster
                    nc.sync.dma_start(
                        x_sb[:].rearrange("p (t d) -> p t d", d=D),
                        x_d.ap()[s * SR:(s + 1) * SR, :].rearrange(
                            "(p t) d -> p t d", p=128),
                    )
                else:
                    for h in range(2):
                        nc.sync.dma_start(
                            x_sb[:, 512 * h:512 * (h + 1)].rearrange(
                                "p (t d) -> p t d", d=D),
                            x_d.ap()[s * SR:(s + 1) * SR, :].rearrange(
                                "(p t) d -> p t d", p=128)[:, 4 * h:4 * (h + 1), :],
                        )
                if s == 0:
                    load_consts()
                elif s == 4:
                    load_half_inputs(0)
                elif s == 6:
                    prep_half(0)
                elif s == 8:
                    load_half_inputs(1)
                elif s == 10:
                    prep_half(1)
            if 2 <= it < NSTRIP + 2:
                s = it - 2
                x_sb = xt_sbs[s][0]
                # xt_sb[d, 128t+p] = row 8p+t; two half-strip psum tiles
                xt_sb = tpool.tile([128, SR], F32R, name=f"xt{s % 5}")
                xt_sbs[s] = (x_sb, xt_sb)
                for h in range(2):
                    xt_ps = ps_xt.tile([128, 512], F32R, name=f"xtp{h}")
                    for tq in range(4):
                        t = 4 * h + tq
                        nc.tensor.transpose(
                            xt_ps[:, tq * 128:(tq + 1) * 128],
                            x_sb[:, t * D:(t + 1) * D],
                            eye_sb[:],
                        )
                    if h == 0:
                        nc.vector.tensor_copy(xt_sb[:, 0:512], xt_ps[:])
                    else:
                        nc.scalar.copy(xt_sb[:, 512:1024], xt_ps[:])
            if 3 <= it < NSTRIP + 3:
                s = it - 3
                xt_sb = xt_sbs.pop(s)[1]
                ht_ps = ps_ht.tile([H, SR], F32)
                for k in range(2):
                    nc.tensor.matmul(
                        ht_ps[:, k * 512:(k + 1) * 512],
                        w1_sb[:],
                        xt_sb[:, k * 512:(k + 1) * 512],
                        start=True, stop=True,
                    )
                # relu + permute to natural row order, packed [128, 512]:
                # relu_sb[64e+h', 8jh+jl] = relu(ht[h', 128jl+64e+jh] + b1)
                relu_sb = rpool.tile([128, 512], F32R, name=f"r{s % 5}")
                relu_sbs[s] = relu_sb
                ht_v = ht_ps[:].rearrange("h (jl pp) -> h pp jl", pp=128)
                nc.vector.tensor_scalar(
                    relu_sb[0:64, :].rearrange("h (jh jl) -> h jh jl", jl=8),
                    ht_v[:, 0:64, :], b1_sb[:, 0:1], 0.0,
                    op0=ALU.add, op1=ALU.max)
                nc.scalar.activation(
                    relu_sb[64:128, :].rearrange("h (jh jl) -> h jh jl", jl=8),
                    ht_v[:, 64:128, :], AF.Relu, bias=b1_sb[:, 0:1])
            if 4 <= it:
                s = it - 4
                hf, sp = s // 32, s % 32
                relu_sb = relu_sbs.pop(s)
                # dl[2*sp+e, j] += w2d . relu_half_e[:, j]
                nc.tensor.matmul(
                    dl_ps[hf][:],
                    w2p_sb[:, 62 - 2 * sp:126 - 2 * sp],
                    relu_sb[:],
                    start=(sp == 0), stop=(sp == 31),
                    skip_group_check=True,
                )
                if s == 31:
                    pending_ops.extend(final_half_ops(0))
                if pending_ops:
                    pending_ops.pop(0)()
        for op in pending_ops:
            op()
        for op in final_half_ops(1):
            op()

    nc.compile()
    nc.m = get_hw_module(nc.m)
    return nc


def kernel(slots, gumbel_u, fix_u, W1, b1, W2, b2, _trace=False):
    slots = np.ascontiguousarray(slots, np.float32)
    gumbel_u = np.ascontiguousarray(gumbel_u, np.float32)
    fix_u = np.ascontiguousarray(fix_u, np.float32)
    W1 = np.ascontiguousarray(W1, np.float32)
    W2 = np.ascontiguousarray(W2, np.float32)
    w2d = (W2[:, 1] - W2[:, 0]).astype(np.float32)
    b2d = np.float32(b2[1] - b2[0])

    w2p = np.zeros((128, 126), np.float32)
    w2p[0:64, 62] = w2d
    w2p[64:128, 63] = w2d
    b1c = np.ascontiguousarray(b1, np.float32).reshape(H, 1)
    b2dv = np.full((H, 1), b2d, np.float32)
    eye = np.eye(128, dtype=np.float32)

    if "nc" not in _CACHE:
        _CACHE["nc"] = _build()
    nc = _CACHE["nc"]

    bpc = B // NCORES
    in_maps = []
    for c in range(NCORES):
        in_maps.append({
            "x": slots[c * bpc:(c + 1) * bpc].reshape(R, D),
            "gu": gumbel_u[c * bpc:(c + 1) * bpc].reshape(R, 2),
            "fu": fix_u[c * bpc:(c + 1) * bpc].reshape(R),
            "w1": W1, "w2p": w2p, "b1c": b1c, "b2dv": b2dv, "eye": eye,
        })
    res = bass_utils.run_bass_kernel_spmd(
        nc, in_maps, core_ids=list(range(NCORES)), trace=_trace)
    _CACHE["last_result"] = res

    dec = np.concatenate(
        [res.results[c]["dec"].reshape(bpc, K) for c in range(NCORES)], axis=0)
    keep = np.concatenate(
        [res.results[c]["keep"].reshape(bpc, K) for c in range(NCORES)], axis=0)
    return dec, keep
